# revision 2
# baseline (speedup 1.0000x reference)
"""2-layer GCN (PyG GCNConv x2, eval mode) on 8 TRN2 NeuronCores, SPMD.

v2: restructured from baseline to cut Pool-engine (SWDGE) descriptor work:
  - self-loops computed locally (no gather descriptors for them)
  - groups of GB=10 dst blocks, NG=10 groups -> 40 gather calls/layer
    (one per (group, src-range)) with ~10k descriptors each
  - trailing pad slots use idx=-1 + num_idxs_reg=valid-count (skipped by DMA)
  - per-group idx streaming (saves ~5.6MB SBUF), per-(g,r) stage/s_all tiles
  - phase A x-transpose loads chunked to overlap with matmuls
"""

import numpy as np
import ml_dtypes

import concourse.bass as bass
import concourse.mybir as mybir
import concourse.tile as tile
import concourse.bacc as bacc
from concourse.bass_utils import run_bass_kernel_spmd

F32 = mybir.dt.float32
BF16 = mybir.dt.bfloat16
I16 = mybir.dt.int16

HID = 128
P = 128
NRANGE = 4
N_NODES = 100000
F_IN = 165
CORES = 8
NB = 100          # dst blocks per core (12800 padded nodes/core)
GB = 5            # dst blocks per group
NSH = 12500
NSH_PAD = NB * P
V_PAD = CORES * NSH_PAD
RANGE = V_PAD // NRANGE
NG = NB // GB
SINGLE_PACKET = False



# --- multi-queue SWDGE sem-lane pinning -------------------------------------
# Tile's sem assigner round-robins the 8 DMASW lanes in scheduled order,
# ignoring queue_num; a sem lane may then be shared by two SWDGE queues,
# which the HW (and CoreSim) reject. Pin lanes per queue: queue q uses
# lanes {2q, 2q+1}.
import concourse.tile_sem_assignment as _tsa
from concourse.tile_scheduler import DMAInst as _DMAInst

if not getattr(_tsa.TileClockTick, "_mq_patched", False):
    _orig_assign_tick = _tsa.TileClockTick._assign_tick

    def _assign_tick_mq(self, inst):
        if (isinstance(inst, _DMAInst)
                and inst.engine == mybir.EngineType.Pool):
            qn = getattr(inst, "queue_num", 0) or 0
            if not hasattr(self, "_mq_counters"):
                self._mq_counters = [0, 0, 0, 0]
            self.next_sw_dma_idx = 2 * qn + (self._mq_counters[qn] % 2)
            self._mq_counters[qn] += 1
        return _orig_assign_tick(self, inst)

    _tsa.TileClockTick._assign_tick = _assign_tick_mq
    _tsa.TileClockTick._mq_patched = True
# ---------------------------------------------------------------------------

def _edge_structure(src, dst):
    """Bucket edges by (dst-core, group, src-range, dl). Returns maxed
    tile grid + per-edge placement keys."""
    csh = np.minimum(dst // NSH, CORES - 1)
    r_loc = dst - csh * NSH
    blk = r_loc // P
    dloc = r_loc % P
    csrc = np.minimum(src // NSH, CORES - 1)
    loc = src - csrc * NSH
    q = loc // (NSH_PAD // NRANGE)
    grow = q * RANGE + csrc * (NSH_PAD // NRANGE) + loc % (NSH_PAD // NRANGE)
    rng = q
    grp = blk // GB
    dl = blk % GB
    key = ((csh * NG + grp) * NRANGE + rng) * GB + dl
    nkeys = CORES * NG * NRANGE * GB
    counts = np.bincount(key, minlength=nkeys).reshape(CORES, NG, NRANGE, GB)
    maxc = counts.max(axis=0)
    tiles_grd = ((maxc + P - 1) // P).astype(np.int64)
    return tiles_grd, key, counts, grow, dloc


def _build_kernel(tiles_grd, t_call, t_group, g_start, valid_counts):
    nc = bacc.Bacc("TRN2", target_bir_lowering=False, debug=False,
                   num_devices=CORES, num_swdge_queues=4)
    NT_TOT = int(g_start[-1])
    TG_MAX = int(t_group.max())
    TR_MAX = int(t_call.max())

    xT_d = nc.dram_tensor("xT", [F_IN, NSH_PAD], F32, kind="ExternalInput")
    w1_d = nc.dram_tensor("W1", [F_IN, HID], F32, kind="ExternalInput")
    b1t_d = nc.dram_tensor("B1T", [P, HID], F32, kind="ExternalInput")
    w2cb_d = nc.dram_tensor("W2CB", [P, 2 * HID], F32, kind="ExternalInput")
    iota_d = nc.dram_tensor("IOTA", [P, P], F32, kind="ExternalInput")
    dinv_d = nc.dram_tensor("DINV", [P, NB], F32, kind="ExternalInput")
    idx_d = nc.dram_tensor("IDX", [P, 8 * NT_TOT], I16, kind="ExternalInput")
    dstloc_d = nc.dram_tensor("DSTLOC", [P, NT_TOT], F32,
                              kind="ExternalInput")
    out_d = nc.dram_tensor("OUT", [P, 2 * NB], F32, kind="ExternalOutput")

    KA = min(F_IN, P)
    KB = F_IN - KA
    XC = 4                      # phase-A column chunks
    CW = NSH_PAD // XC          # 3200 nodes per chunk

    with tile.TileContext(nc) as tc:
        with (
            tc.tile_pool(name="const", bufs=1) as cpool,
            tc.tile_pool(name="dram", bufs=1, space="DRAM") as dpool,
        ):
            iota_sb = cpool.tile([P, P], F32)
            dinv_sb = cpool.tile([P, NB], F32)
            b1t_sb = cpool.tile([P, HID], F32)
            w2cb_sb = cpool.tile([P, 2 * HID], F32)
            outsb = cpool.tile([P, 2 * NB], F32)
            g1sb = cpool.tile([P, NB * HID], BF16)   # own-shard g1 rows
            g2sb = cpool.tile([P, NB * HID], BF16)   # own-shard g2 rows
            nc.sync.dma_start(out=iota_sb[:], in_=iota_d[:, :])
            nc.sync.dma_start(out=dinv_sb[:], in_=dinv_d[:, :])
            nc.sync.dma_start(out=b1t_sb[:], in_=b1t_d[:, :])
            nc.sync.dma_start(out=w2cb_sb[:], in_=w2cb_d[:, :])

            sh1 = dpool.tile([NSH_PAD, HID], BF16)
            sh2 = dpool.tile([NSH_PAD, HID], BF16)
            tb1 = [dpool.tile([RANGE, HID], BF16, addr_space="Shared",
                              name=f"tb1q{q}") for q in range(NRANGE)]
            tb2 = [dpool.tile([RANGE, HID], BF16, addr_space="Shared",
                              name=f"tb2q{q}") for q in range(NRANGE)]

            # Phase A: g1 = (x @ W1) * dinv -> sh1 (DRAM) + g1sb (SBUF)
            with (
                tc.tile_pool(name="mm_w", bufs=1) as wpool,
                tc.tile_pool(name="mm_x", bufs=2) as xpool,
                tc.tile_pool(name="mm_ps", bufs=4, space="PSUM") as pspool,
                tc.tile_pool(name="mm_g", bufs=4) as gpool,
            ):
                w1a = wpool.tile([KA, HID], F32)
                nc.sync.dma_start(out=w1a[:], in_=w1_d[0:KA, :])
                w1b = wpool.tile([KB, HID], F32)
                nc.sync.dma_start(out=w1b[:], in_=w1_d[KA:F_IN, :])
                for ch in range(XC):
                    c0 = ch * CW
                    xta = xpool.tile([KA, CW], F32, tag="xa")
                    nc.sync.dma_start(out=xta[:], in_=xT_d[0:KA, c0:c0 + CW])
                    xtb = xpool.tile([KB, CW], F32, tag="xb")
                    nc.sync.dma_start(out=xtb[:],
                                      in_=xT_d[KA:F_IN, c0:c0 + CW])
                    for dd in range(CW // P):
                        d = ch * (CW // P) + dd
                        ps = pspool.tile([P, HID], F32, space="PSUM",
                                         tag="ps_a")
                        sl = slice(dd * P, (dd + 1) * P)
                        nc.tensor.matmul(out=ps[:], lhsT=xta[:, sl],
                                         rhs=w1a[:], start=True, stop=False)
                        nc.tensor.matmul(out=ps[:], lhsT=xtb[:, sl],
                                         rhs=w1b[:], start=False, stop=True)
                        g1 = gpool.tile([P, HID], BF16, tag="g_a")
                        nc.vector.tensor_scalar(out=g1[:], in0=ps[:],
                                                scalar1=dinv_sb[:, d:d + 1],
                                                scalar2=None,
                                                op0=mybir.AluOpType.mult)
                        nc.scalar.copy(out=g1sb[:, d * HID:(d + 1) * HID],
                                       in_=g1[:])
                        nc.sync.dma_start(out=sh1[d * P:(d + 1) * P, :],
                                          in_=g1[:])

            QSH = NSH_PAD // NRANGE
            for q in range(NRANGE):
                nc.gpsimd.collective_compute(
                    "AllGather", mybir.AluOpType.bypass,
                    replica_groups=[list(range(CORES))],
                    ins=[sh1[q * QSH:(q + 1) * QSH, :].opt()],
                    outs=[tb1[q][:].opt()],
                )

            def run_layer(table, layer):
                with (
                    tc.tile_pool(name=f"ix{layer}", bufs=2) as ixpool,
                    tc.tile_pool(name=f"dl{layer}", bufs=2) as dlpool,
                    tc.tile_pool(name=f"st{layer}", bufs=2) as stpool,
                    tc.tile_pool(name=f"s{layer}", bufs=1) as spool,
                    tc.tile_pool(name=f"ps{layer}", bufs=4,
                                 space="PSUM") as pspool,
                    tc.tile_pool(name=f"z{layer}", bufs=4) as zpool,
                    tc.tile_pool(name=f"pb{layer}", bufs=4) as pbpool,
                    tc.tile_pool(name=f"t2{layer}", bufs=4) as tmppool,
                ):
                    for g in range(NG):
                        gs = int(g_start[g])
                        tg = int(t_group[g])
                        ix = ixpool.tile([P, 8 * TG_MAX], I16, tag="ix")
                        nc.sync.dma_start(out=ix[:, 0:8 * tg],
                                          in_=idx_d[:, 8 * gs:8 * (gs + tg)])
                        dsl = dlpool.tile([P, TG_MAX], F32, tag="dsl")
                        nc.sync.dma_start(out=dsl[:, 0:tg],
                                          in_=dstloc_d[:, gs:gs + tg])
                        stages = {}
                        roff = 0
                        for r in range(NRANGE):
                            tr = int(t_call[g][r])
                            if tr == 0:
                                continue
                            stg = stpool.tile([P, TR_MAX * HID], BF16,
                                              tag=f"stage{r}",
                                              name=f"stage_{r}")
                            stages[r] = stg
                            nc.gpsimd.dma_gather(
                                out_ap=stg[:, 0:tr * HID]
                                    .rearrange("p (t j) -> p t j", j=HID),
                                in_ap=table[r][0:RANGE, :],
                                idxs_ap=ix[:, 8 * roff:8 * (roff + tr)],
                                num_idxs=tr * P,
                                num_idxs_reg=tr * P,
                                elem_size=HID,
                                single_packet=SINGLE_PACKET,
                                queue_num=r)
                            roff += tr
                        s_all = spool.tile([P, TG_MAX * P], BF16, tag="s")
                        nc.vector.tensor_tensor(
                            out=s_all[:, 0:tg * P].rearrange(
                                "p (t j) -> p t j", j=P),
                            in0=iota_sb[:].unsqueeze(1)
                                .to_broadcast([P, tg, P]),
                            in1=dsl[:, 0:tg].unsqueeze(2)
                                .to_broadcast([P, tg, P]),
                            op=mybir.AluOpType.is_equal)
                        for dl in range(GB):
                            d = g * GB + dl
                            njobs = int(tiles_grd[g, :, dl].sum())
                            gslice = slice(d * HID, (d + 1) * HID)
                            z = zpool.tile([P, HID], F32, tag="z")
                            own = g1sb if layer == 1 else g2sb
                            if njobs > 0:
                                ps = pspool.tile([P, HID], F32, space="PSUM",
                                                 tag="ps")
                                done = 0
                                roff = 0
                                for r in range(NRANGE):
                                    base = roff + int(
                                        tiles_grd[g, r, :dl].sum())
                                    bloc = int(tiles_grd[g, r, :dl].sum())
                                    for t in range(int(tiles_grd[g, r, dl])):
                                        c = base + t
                                        cl = bloc + t
                                        nc.tensor.matmul(
                                            out=ps[:],
                                            lhsT=s_all[:, c * P:(c + 1) * P],
                                            rhs=stages[r][:,
                                                          cl * HID:
                                                          (cl + 1) * HID],
                                            start=(done == 0),
                                            stop=(done == njobs - 1))
                                        done += 1
                                    roff += int(t_call[g][r])
                                nc.vector.tensor_tensor(
                                    out=z[:], in0=ps[:], in1=own[:, gslice],
                                    op=mybir.AluOpType.add)
                            else:
                                nc.vector.tensor_scalar_add(
                                    out=z[:], in0=own[:, gslice],
                                    scalar1=0.0)
                            nc.vector.tensor_scalar(
                                out=z[:], in0=z[:],
                                scalar1=dinv_sb[:, d:d + 1],
                                scalar2=None,
                                op0=mybir.AluOpType.mult)
                            if layer == 1:
                                nc.vector.tensor_tensor(
                                    out=z[:], in0=z[:], in1=b1t_sb[:],
                                    op=mybir.AluOpType.add)
                                p_bf = pbpool.tile([P, HID], BF16, tag="pb")
                                nc.scalar.activation(
                                    out=p_bf[:], in_=z[:],
                                    func=mybir.ActivationFunctionType.Relu,
                                    scale=dinv_sb[:, d:d + 1])
                                nc.scalar.copy(out=g2sb[:, gslice],
                                               in_=p_bf[:])
                                nc.sync.dma_start(
                                    out=sh2[d * P:(d + 1) * P, :],
                                    in_=p_bf[:])
                            else:
                                for chn in range(2):
                                    tmp = tmppool.tile([P, HID], F32,
                                                       tag="t2")
                                    nc.vector.tensor_tensor(
                                        out=tmp[:], in0=z[:],
                                        in1=w2cb_sb[:,
                                                    chn * HID:(chn + 1) * HID],
                                        op=mybir.AluOpType.mult)
                                    nc.vector.tensor_reduce(
                                        out=outsb[:,
                                                  2 * d + chn:2 * d + chn + 1],
                                        in_=tmp[:], axis=mybir.AxisListType.X,
                                        op=mybir.AluOpType.add)

            run_layer(tb1, 1)

            for q in range(NRANGE):
                nc.gpsimd.collective_compute(
                    "AllGather", mybir.AluOpType.bypass,
                    replica_groups=[list(range(CORES))],
                    ins=[sh2[q * QSH:(q + 1) * QSH, :].opt()],
                    outs=[tb2[q][:].opt()],
                )

            run_layer(tb2, 2)

            nc.sync.dma_start(out=out_d[:, :], in_=outsb[:])

    nc.compile()
    return nc


def _prep(x, edge_index, W1, b1, W2):
    src = np.asarray(edge_index[0], dtype=np.int64)
    dst = np.asarray(edge_index[1], dtype=np.int64)

    deg = np.bincount(dst, minlength=N_NODES) + 1   # + self loop
    dinv = np.where(deg > 0, 1.0 / np.sqrt(deg.astype(np.float64)),
                    0.0).astype(np.float32)

    tiles_grd, key, counts, grow, dloc = _edge_structure(src, dst)

    t_call = tiles_grd.sum(axis=2)              # [NG, NRANGE]
    t_group = t_call.sum(axis=1)                # [NG]
    g_start = np.concatenate([[0], np.cumsum(t_group)])
    NT_TOT = int(g_start[-1])

    # per-(g,r): tile offset of bucket dl within the (g,r) call
    buck_base = np.zeros((NG, NRANGE, GB), np.int64)
    for g in range(NG):
        for r in range(NRANGE):
            acc = 0
            for dl in range(GB):
                buck_base[g, r, dl] = acc
                acc += int(tiles_grd[g, r, dl])
    # column offset (in tiles) of call (g,r) within group g
    call_off = np.zeros((NG, NRANGE), np.int64)
    for g in range(NG):
        acc = 0
        for r in range(NRANGE):
            call_off[g, r] = acc
            acc += int(t_call[g, r])

    order = np.argsort(key, kind="stable")
    starts = np.concatenate([[0], np.cumsum(counts.reshape(-1))])

    idx_all = np.zeros((CORES, P, 8 * NT_TOT), np.int16)
    dst_all = np.full((CORES, P, NT_TOT), -1.0, np.float32)
    # valid slot count per (core, g, r): max J over buckets (+1)
    valid_all = np.zeros((CORES, NG, NRANGE), np.int64)
    kflat = 0
    for c in range(CORES):
        for g in range(NG):
            for r in range(NRANGE):
                for dl in range(GB):
                    s0, s1 = starts[kflat], starts[kflat + 1]
                    kflat += 1
                    cnt = s1 - s0
                    if cnt == 0:
                        continue
                    e = order[s0:s1]
                    J = buck_base[g, r, dl] * P + np.arange(cnt)
                    gt = g_start[g] + call_off[g, r] + J // P
                    dst_all[c, J % P, gt] = dloc[e].astype(np.float32)
                    icol = 8 * (call_off[g, r]) + J // 16
                    idx_all[c, J % 16, 8 * g_start[g] + icol] = (
                        grow[e] - r * RANGE).astype(np.int16)
                    valid_all[c, g, r] = max(valid_all[c, g, r],
                                             int(J[-1]) + 1)
    for q in range(1, 8):
        idx_all[:, 16 * q:16 * (q + 1), :] = idx_all[:, 0:16, :]

    valid_max = valid_all.max(axis=0)

    dinv_pb = np.zeros((CORES, P, NB), np.float32)
    for c in range(CORES):
        n0, n1 = c * NSH, min((c + 1) * NSH, N_NODES)
        loc = np.zeros(NSH_PAD, np.float32)
        loc[: n1 - n0] = dinv[n0:n1]
        dinv_pb[c] = loc.reshape(NB, P).T

    xT = np.ascontiguousarray(np.asarray(x, np.float32).T)
    b1t = np.tile(np.asarray(b1, np.float32)[None, :], (P, 1))
    w2 = np.asarray(W2, np.float32)
    w2cb = np.zeros((P, 2 * HID), np.float32)
    for chn in range(2):
        w2cb[:, chn * HID:(chn + 1) * HID] = np.tile(w2[:, chn][None, :],
                                                     (P, 1))
    iota = np.tile(np.arange(P, dtype=np.float32)[None, :], (P, 1))
    W1f = np.asarray(W1, np.float32)

    in_maps = []
    for c in range(CORES):
        n0, n1 = c * NSH, min((c + 1) * NSH, N_NODES)
        xtc = np.zeros((F_IN, NSH_PAD), np.float32)
        xtc[:, : n1 - n0] = xT[:, n0:n1]
        in_maps.append({
            "xT": xtc, "W1": W1f, "B1T": b1t, "W2CB": w2cb, "IOTA": iota,
            "DINV": dinv_pb[c], "IDX": idx_all[c], "DSTLOC": dst_all[c],
        })
    return (tiles_grd, t_call, t_group, g_start, valid_max), in_maps


def kernel(x, edge_index, W1, b1, W2, b2):
    import os
    x = np.asarray(x)
    edge_index = np.asarray(edge_index)
    W1 = np.asarray(W1)
    b1 = np.asarray(b1)
    W2 = np.asarray(W2)
    b2 = np.asarray(b2, dtype=np.float32)
    assert x.shape == (N_NODES, F_IN), x.shape

    meta, in_maps = _prep(x, edge_index, W1, b1, W2)
    tiles_grd, t_call, t_group, g_start, valid_max = meta
    nc = _build_kernel(tiles_grd, t_call, t_group, g_start, valid_max)
    trace = bool(int(os.environ.get("GCN_TRACE", "0")))
    try:
        res = run_bass_kernel_spmd(nc, in_maps, core_ids=list(range(CORES)),
                                   trace=trace)
    except Exception:
        if not trace:
            raise
        res = run_bass_kernel_spmd(nc, in_maps, core_ids=list(range(CORES)),
                                   trace=False)
    if trace and res.exec_time_ns is not None:
        print(f"HW exec time: {res.exec_time_ns} ns")

    out = np.zeros((N_NODES, 2), np.float32)
    for c in range(CORES):
        buf = res.results[c]["OUT"]
        arr = buf.reshape(P, NB, 2).transpose(1, 0, 2).reshape(NSH_PAD, 2)
        n0, n1 = c * NSH, min((c + 1) * NSH, N_NODES)
        out[n0:n1] = arr[: n1 - n0]
    return out + b2[None, :]
